# revision 20
# baseline (speedup 1.0000x reference)
"""MixHopNet GNN kernel for 8 Trainium2 NeuronCores (Bass/Tile SPMD).

Math (reference): GCN-normalized adjacency A = D^-1/2 (Adj + I) D^-1/2 over
N=50000 nodes / 800k random edges (+self loops), then
  x1 = A x ; x2 = A x1
  h  = relu([x w1_0 + b1_0, x1 w1_1 + b1_1, x2 w1_2 + b1_2])
  out = log_softmax([h w2_0 + b2_0, (A h) w2_1 + b2_1])

Distribution (graph/data parallel): nodes are packed into 456 blocks of 128
slots, 57 blocks per core.  Propagation for a dst block accumulates
S_j^T @ V_j over edge chunks j of 128 edges in PSUM, where S_j is a 0/1
one-hot (edge -> dst slot) PRECOMPUTED ON HOST, streamed from HBM as
fp8_e4m3 (1.0 is exact), with an identity prefix chunk per block that
applies the reference's self loop.

P1 (x1 = A x): the table is known on host, so V is HOST-PREGATHERED and
streamed sequentially (no dma_gather descriptors, no on-chip one-hot
builds).  P2/P3 V is dma_gather'd from the bf16 table; SWDGE descriptor
generation on the GpSimd engine (~2ns/desc serial) is the critical
resource, so gathers are GROUPED (G blocks x piece per call) to amortize
per-call overhead, round-robin over the 4 SWDGE queues.

dma_gather needs int16 row indices, so each table is split in two pieces
by the source's block POSITION within its core (< PA_BLK or not).  The
pieces are AllGather'd SEPARATELY: the piece-a collective is issued
mid-propagation (after the first PA_BLK blocks evict), hiding most of its
latency under the tail of the producing propagation.

Softmax head avoids DVE tensor_scalar with per-partition scalars (~4.5us
each on HW): t1n = -(logits+bias) via tensor_tensor, mxn = reduce_min,
ACT fuses shift+exp as Exp(-t1n + mxn) with accum_out, and the final
subtract is ACT Identity(-t1n + (mxn - lse)).  The Ln over row-sums runs
in 3 batches so most output stores overlap the P3 gather loop (each
Exp<->Ln function-table switch costs ~1.3us on ACT).
"""
import sys

sys.path.insert(0, "/opt/trn_rl_repo")

import numpy as np

import concourse.bass as bass  # noqa: F401
import concourse.bacc as bacc
import concourse.tile as tile
from concourse import mybir
from concourse.bass_utils import run_bass_kernel_spmd

import ml_dtypes

BF16 = ml_dtypes.bfloat16
FP8 = ml_dtypes.float8_e4m3

# ---- problem constants (hardcoded; kernel.py must be self-contained) ----
N = 50000
FIN = 128
H = 128
CO = 40
NCORES = 8
P = 128
NB = 456               # node blocks total
BPC = NB // NCORES     # 57 blocks per core
S = NB * P             # 58368 slots
NSH = BPC * P          # 7296 slots per core
NHALF = N // 2
PA_BLK = 28            # piece-a blocks per core (chunked-AG split)
PA_ROWS = PA_BLK * P   # 3584
PB_ROWS = NSH - PA_ROWS
SA = NCORES * PA_ROWS  # 28672 rows in the piece-a table (int16-addressable)
SB = NCORES * PB_ROWS  # 29696
NQ = 4                 # SWDGE queues
G = 1                  # blocks per gather group
AG_EMIT = 34           # emit piece-a AllGather before this block's streams

LAST_RESULT = None     # BassKernelResults of the most recent run (for test.py)

_COMPILED = {}


# --------------------------------------------------------------------------
# host-side preprocessing
# --------------------------------------------------------------------------
def _pack_nodes(a, b):
    """Assign each node a slot: nodes [0,NHALF) -> blocks [0,NB/2), rest ->
    blocks [NB/2,NB).  Greedy 2D bin packing (descending total degree,
    minimize max load) balances block sizes."""
    node2slot = np.empty(N, np.int64)
    for hstart, hend, b0 in ((0, NHALF, 0), (NHALF, N, NB // 2)):
        nbins = NB // 2
        nodes = np.arange(hstart, hend)
        nodes = nodes[np.argsort(-(a[nodes] + b[nodes]), kind="stable")]
        lo_load = np.zeros(nbins, np.int64)
        hi_load = np.zeros(nbins, np.int64)
        cnt = np.zeros(nbins, np.int64)
        av = a[nodes]
        bv = b[nodes]
        for i in range(nodes.shape[0]):
            score = np.maximum(lo_load + av[i], hi_load + bv[i])
            score[cnt >= P] = 1 << 60
            blk = int(np.argmin(score))
            node2slot[nodes[i]] = (b0 + blk) * P + cnt[blk]
            cnt[blk] += 1
            lo_load[blk] += av[i]
            hi_load[blk] += bv[i]
    return node2slot


def _wrap_idx(flat):
    """[n*128] int16 -> [128, n*8] (16-partition wrap, replicated 8x)."""
    n = flat.shape[0] // 128
    arr = flat.reshape(n * 8, 16).T.copy()
    return np.tile(arr, (8, 1))


def _preprocess(x, edge_index, w1_0, b1_0, w1_1, b1_1, w1_2, b1_2,
                w2_0, b2_0, w2_1, b2_1):
    src = edge_index[0].astype(np.int64)
    dst = edge_index[1].astype(np.int64)
    E = src.shape[0]

    deg = (np.bincount(dst, minlength=N) + 1).astype(np.float32)  # +self loop
    dinv = (1.0 / np.sqrt(deg)).astype(np.float32)

    islo_n = src < NHALF
    a = np.bincount(dst[islo_n], minlength=N)
    b = np.bincount(dst[~islo_n], minlength=N)
    node2slot = _pack_nodes(a, b)

    # deal blocks to cores: snake by total edges so cores get similar
    # block-size profiles (position k has ~equal size on every core).
    blk_old = node2slot[dst] >> 7
    tot = np.bincount(blk_old, minlength=NB)
    order_bs = np.empty(NB, np.int64)
    blk_perm = np.empty(NB, np.int64)
    for half, coff in ((0, 0), (1, 4)):
        ids = np.arange(half * (NB // 2), (half + 1) * (NB // 2))
        ids = ids[np.argsort(-tot[ids], kind="stable")]
        percore = [[] for _ in range(4)]
        for i, bid in enumerate(ids):
            c = i % 4 if (i // 4) % 2 == 0 else 3 - i % 4
            percore[c].append(bid)
        for c in range(4):
            for j, bid in enumerate(percore[c]):
                pos = (coff + c) * BPC + j
                order_bs[pos] = bid
                blk_perm[bid] = pos
    slot_perm = (blk_perm[:, None] * P + np.arange(P)[None, :]).reshape(-1)
    node2slot = slot_perm[node2slot]

    dslot = node2slot[dst]
    nblk = dslot >> 7                  # global block position (0..NB-1)
    d_in_blk = dslot & 127
    sslot = node2slot[src]

    # piece membership by the src's block position within its core
    srow = sslot % NSH
    score_core = sslot // NSH
    in_a = srow < PA_ROWS
    prow = np.where(in_a, score_core * PA_ROWS + srow,
                    score_core * PB_ROWS + (srow - PA_ROWS))

    # chunk counts per (within-core position, piece): max across cores
    grp = nblk * 2 + (~in_a).astype(np.int64)
    cnts = np.bincount(grp, minlength=2 * NB).reshape(NB, 2)
    ch = np.ceil(cnts / P).astype(np.int64).reshape(NCORES, BPC, 2)
    K_a = np.maximum(ch[:, :, 0].max(axis=0), 1)  # [BPC]
    K_b = np.maximum(ch[:, :, 1].max(axis=0), 1)
    Ktot = K_a + K_b
    TOTCH = int(Ktot.sum())
    PCH = TOTCH + BPC                  # + identity prefix chunk per block

    # canonical chunk order: per block [a chunks, b chunks]
    base_a = np.concatenate([[0], np.cumsum(Ktot)[:-1]])
    base_b = base_a + K_a

    # flatten edges into the padded chunk layout
    order = np.argsort(grp, kind="stable")
    gs = np.bincount(grp, minlength=2 * NB)
    starts = np.concatenate([[0], np.cumsum(gs)[:-1]])
    epos = np.arange(E) - starts[grp[order]]
    posc = np.arange(NB) % BPC
    gbase = np.stack([base_a[posc], base_b[posc]], axis=1)
    eslot = gbase[nblk[order], (~in_a[order]).astype(np.int64)] * P + epos
    core_e = nblk[order] // BPC

    sidx_pad = np.zeros((NCORES, TOTCH * P), np.int16)
    sidx_pad[core_e, eslot] = prow[order].astype(np.int16)
    gslot_pad = np.zeros((NCORES, TOTCH * P), np.int64)
    gslot_pad[core_e, eslot] = sslot[order]

    # gather-call layout: per group of G blocks, per piece, the chunks of
    # those blocks in block order; idx stream follows this order.
    NG = (BPC + G - 1) // G
    chunk_order = []
    for g in range(NG):
        blks = range(g * G, min((g + 1) * G, BPC))
        for base, K in ((base_a, K_a), (base_b, K_b)):
            for bb in blks:
                chunk_order.extend(range(int(base[bb]),
                                         int(base[bb] + K[bb])))
    chunk_order = np.asarray(chunk_order)
    idx_maps = [
        _wrap_idx(sidx_pad[c].reshape(TOTCH, P)[chunk_order].reshape(-1))
        for c in range(NCORES)]

    # one-hot stream, fp8, canonical order with identity prefix per block
    pbase = base_a + np.arange(BPC)
    ident8 = np.eye(P, dtype=FP8)
    one8 = np.float32(1.0).astype(FP8)
    oh8 = np.zeros((NCORES, PCH, P, P), FP8)
    ch_of_edge = eslot >> 7
    blk_of_ch = np.repeat(np.arange(BPC), Ktot)
    pch_of_edge = ch_of_edge + blk_of_ch[ch_of_edge] + 1
    oh8[core_e, pch_of_edge, eslot & 127, d_in_blk[order]] = one8
    oh8[:, pbase, :, :] = ident8[None, None]

    dinv_slot = np.zeros(S, np.float32)
    dinv_slot[node2slot] = dinv
    x_slot = np.zeros((S, FIN), np.float32)
    x_slot[node2slot] = x
    u0 = (x_slot * dinv_slot[:, None]).astype(BF16)

    brow_neg = np.tile(-np.concatenate([b2_0, b2_1])[None, :],
                       (P, 1)).astype(np.float32)
    w1s = np.concatenate([w1_0, w1_1, w1_2], axis=1).astype(BF16)
    b1m = np.stack([b1_0, b1_1, b1_2], axis=1).astype(np.float32)
    # concat heads: [out1 (w2_0) | z1 (w2_1)] per input layer
    w2_0 = np.asarray(w2_0, np.float32)
    w2_1 = np.asarray(w2_1, np.float32)
    wcat = np.concatenate([w2_0, w2_1], axis=1).astype(BF16)  # [384, 80]
    ident = np.eye(P, dtype=BF16)
    is_pref = np.zeros(PCH, bool)
    is_pref[pbase] = True

    in_maps = []
    for c in range(NCORES):
        rows = slice(c * NSH, (c + 1) * NSH)
        dm = dinv_slot[rows].reshape(BPC, P).T.copy()
        vg = np.empty((PCH, P, FIN), BF16)
        vg[~is_pref] = u0[gslot_pad[c]].reshape(TOTCH, P, FIN)
        vg[pbase] = u0[rows].reshape(BPC, P, FIN)
        vgc = np.ascontiguousarray(vg.transpose(1, 0, 2).reshape(P, PCH * FIN))
        ohc = np.ascontiguousarray(
            oh8[c].transpose(1, 0, 2).reshape(P, PCH * P))
        in_maps.append(dict(
            xT=np.ascontiguousarray(x_slot[rows].T).astype(BF16),
            idx=idx_maps[c],
            oh=ohc,
            vg=vgc,
            dinvc=dm,
            dinv2c=(dm * dm),
            ident=ident,
            w1s=w1s,
            b1m=b1m,
            wcat=wcat,
            brow_neg=brow_neg,
        ))
    return in_maps, node2slot, tuple(int(v) for v in K_a), \
        tuple(int(v) for v in K_b)


# --------------------------------------------------------------------------
# device program
# --------------------------------------------------------------------------
def _build(nc, K_a, K_b):
    dt = mybir.dt
    f32 = dt.float32
    bf16 = dt.bfloat16
    fp8 = dt.float8e4
    Ktot = [a + b for a, b in zip(K_a, K_b)]
    TOTCH = sum(Ktot)
    PCH = TOTCH + BPC
    KT_M = max(Ktot)
    cbase = [0]
    for k in Ktot:
        cbase.append(cbase[-1] + k)
    pbase = [cbase[b] + b for b in range(BPC)]   # stream position of prefix

    # gather-call layout (must match host chunk_order)
    NG = (BPC + G - 1) // G
    groups = [list(range(g * G, min((g + 1) * G, BPC))) for g in range(NG)]
    call_off = []      # [(off_a, na, off_b, nb)] per group, chunk units
    o = 0
    for blks in groups:
        na = sum(K_a[b] for b in blks)
        nb = sum(K_b[b] for b in blks)
        call_off.append((o, na, o + na, nb))
        o += na + nb
    GK_A = max(sum(K_a[b] for b in blks) for blks in groups)
    GK_B = max(sum(K_b[b] for b in blks) for blks in groups)

    xT = nc.dram_tensor("xT", [P, NSH], bf16, kind="ExternalInput").ap()
    idx = nc.dram_tensor("idx", [P, TOTCH * 8], dt.int16,
                         kind="ExternalInput").ap()
    ohd = nc.dram_tensor("oh", [P, PCH * P], fp8, kind="ExternalInput").ap()
    vgd = nc.dram_tensor("vg", [P, PCH * FIN], bf16, kind="ExternalInput").ap()
    dinvc = nc.dram_tensor("dinvc", [P, BPC], f32, kind="ExternalInput").ap()
    dinv2c = nc.dram_tensor("dinv2c", [P, BPC], f32, kind="ExternalInput").ap()
    identd = nc.dram_tensor("ident", [P, P], bf16, kind="ExternalInput").ap()
    w1s = nc.dram_tensor("w1s", [P, 3 * H], bf16, kind="ExternalInput").ap()
    b1m = nc.dram_tensor("b1m", [P, 3], f32, kind="ExternalInput").ap()
    wcat = nc.dram_tensor("wcat", [3 * H, 2 * CO], bf16,
                          kind="ExternalInput").ap()
    brow_neg = nc.dram_tensor("brow_neg", [P, 2 * CO], f32,
                              kind="ExternalInput").ap()
    out = nc.dram_tensor("out", [NSH, 2 * CO], f32, kind="ExternalOutput").ap()

    rg = [list(range(NCORES))]

    with tile.TileContext(nc) as tc:
        with (
            tc.tile_pool(name="res", bufs=1) as res,
            tc.tile_pool(name="dram", bufs=1, space="DRAM") as dram,
        ):
            def load(name, src_ap, shape, dtype=f32, eng=nc.scalar):
                t = res.tile(shape, dtype, tag=name, name=name)
                eng.dma_start(out=t[:], in_=src_ap)
                return t

            # small tiles via ACT's HWDGE queue; idx via the mainline SWDGE
            # queue (GpSimd is idle during P1); vg streams own the SP queue.
            dinvc_t = load("dinvc", dinvc[:], [P, BPC])
            dinv2c_t = load("dinv2c", dinv2c[:], [P, BPC])
            ident_t = load("ident", identd[:], [P, P], bf16)
            w1_t = load("w1s", w1s[:], [P, 3 * H], bf16)
            b1_t = load("b1m", b1m[:], [P, 3])
            brow_t = load("brow_neg", brow_neg[:], [P, 2 * CO])
            wc_t = [load(f"wcat{i}", wcat[i * H:(i + 1) * H, :],
                         [P, 2 * CO], bf16) for i in range(3)]
            idx_t = load("idx", idx[:], [P, TOTCH * 8], dt.int16,
                         eng=nc.gpsimd)

            x1T = res.tile([P, NSH], bf16, tag="x1T")
            x2T = res.tile([P, NSH], bf16, tag="x2T")
            hT = [res.tile([P, NSH], bf16, tag=f"hT{i}", name=f"hT{i}")
                  for i in range(3)]
            # work80[b] holds, in sequence: [out1+z1 partials] during P2,
            # [out1 | z1] after mid, [out1 | out2] in P3, then t1n in place.
            work80 = res.tile([P, BPC * 2 * CO], f32, tag="work80")

            u1b_a = dram.tile([PA_ROWS, FIN], bf16, tag="u1b_a")
            u1b_b = dram.tile([PB_ROWS, FIN], bf16, tag="u1b_b")
            u1f_a = dram.tile([SA, FIN], bf16, tag="u1f_a", addr_space="Shared")
            u1f_b = dram.tile([SB, FIN], bf16, tag="u1f_b", addr_space="Shared")
            uzb_a = dram.tile([PA_ROWS, P], bf16, tag="uzb_a")
            uzb_b = dram.tile([PB_ROWS, P], bf16, tag="uzb_b")
            uzf_a = dram.tile([SA, P], bf16, tag="uzf_a", addr_space="Shared")
            uzf_b = dram.tile([SB, P], bf16, tag="uzf_b", addr_space="Shared")

            def own_rows(ta, tb, b):
                return (ta[b * P:(b + 1) * P, :] if b < PA_BLK
                        else tb[(b - PA_BLK) * P:(b - PA_BLK + 1) * P, :])

            qn = [0]  # SWDGE queue round-robin
            A_LOOK = 16  # piece-a gather calls issued ahead (hide AG_b)

            def prop(tbl_a, tbl_b, own_a, own_b, width, evict, pools):
                """Per-block gather propagation (P2/P3).  Piece-a calls are
                emitted A_LOOK blocks ahead: their desc-gen runs on the Pool
                engine while the piece-b AllGather is still in flight (the
                first piece-b gather head-of-line blocks the engine on it)."""
                pwa, pwb, pp, ohp, sp = pools
                va_t = {}

                def call_a(b):
                    va = pwa.tile([P, GK_A, FIN], bf16, tag="va", name="va")
                    nc.gpsimd.dma_gather(
                        va[:, 0:K_a[b], :], tbl_a,
                        idx_t[:, call_off[b][0] * 8:
                              (call_off[b][0] + K_a[b]) * 8],
                        num_idxs=K_a[b] * P, num_idxs_reg=K_a[b] * P,
                        elem_size=FIN, queue_num=qn[0])
                    qn[0] = (qn[0] + 1) % NQ
                    va_t[b] = va

                for b in range(min(A_LOOK, BPC)):
                    call_a(b)
                for b in range(BPC):
                    va = va_t.pop(b)
                    off_b, nbch = call_off[b][2], K_b[b]
                    vb = pwb.tile([P, GK_B, FIN], bf16, tag="vb", name="vb")
                    nc.gpsimd.dma_gather(
                        vb[:, 0:nbch, :], tbl_b,
                        idx_t[:, off_b * 8:(off_b + nbch) * 8],
                        num_idxs=nbch * P, num_idxs_reg=nbch * P,
                        elem_size=FIN, queue_num=qn[0])
                    qn[0] = (qn[0] + 1) % NQ
                    if b + A_LOOK < BPC:
                        call_a(b + A_LOOK)
                    oht = ohp.tile([P, (KT_M + 1) * P], fp8, tag="oht")
                    nc.scalar.dma_start(
                        out=oht[:, 0:(Ktot[b] + 1) * P],
                        in_=ohd[:, pbase[b] * P:(pbase[b] + Ktot[b] + 1) * P])
                    sblk = sp.tile([P, FIN], bf16, tag="sblk")
                    nc.sync.dma_start(out=sblk[:],
                                      in_=own_rows(own_a, own_b, b))
                    ps = pp.tile([P, width], f32, tag="agg")
                    nc.tensor.matmul(out=ps[:], lhsT=oht[:, 0:P],
                                     rhs=sblk[:, 0:width],
                                     start=True, stop=False)
                    for j in range(Ktot[b]):
                        srcv = (va[:, j, 0:width] if j < K_a[b]
                                else vb[:, j - K_a[b], 0:width])
                        nc.tensor.matmul(
                            out=ps[:],
                            lhsT=oht[:, (j + 1) * P:(j + 2) * P],
                            rhs=srcv,
                            start=False, stop=(j == Ktot[b] - 1))
                    evict(b, ps)

            # ================= P1: x1 = A x (streamed, no gathers) ==========
            with (
                tc.tile_pool(name="p1v", bufs=3) as vp,
                tc.tile_pool(name="p1p", bufs=4, space="PSUM") as pp,
                tc.tile_pool(name="p1o", bufs=3) as ohp,
                tc.tile_pool(name="p1e", bufs=3) as evp,
                tc.tile_pool(name="p1t", bufs=2, space="PSUM") as tpp,
            ):
                def evict1(b, ps):
                    x1t = evp.tile([P, P], bf16, tag="x1t")
                    nc.scalar.mul(x1t[:], ps[:], dinvc_t[:, b:b + 1])
                    u1t = evp.tile([P, P], bf16, tag="u1t")
                    nc.scalar.mul(u1t[:], ps[:], dinv2c_t[:, b:b + 1])
                    nc.sync.dma_start(out=own_rows(u1b_a, u1b_b, b),
                                      in_=u1t[:])
                    trp = tpp.tile([P, P], bf16, tag="trp")
                    nc.tensor.transpose(out=trp[:], in_=x1t[:],
                                        identity=ident_t[:])
                    nc.vector.tensor_copy(out=x1T[:, b * P:(b + 1) * P],
                                          in_=trp[:])

                for b in range(BPC):
                    if b == AG_EMIT:
                        nc.gpsimd.collective_compute(
                            "AllGather", mybir.AluOpType.bypass,
                            replica_groups=rg,
                            ins=[u1b_a.opt()], outs=[u1f_a.opt()])
                    nch = Ktot[b] + 1
                    vg = vp.tile([P, (KT_M + 1) * FIN], bf16, tag="vg")
                    nc.sync.dma_start(
                        out=vg[:, 0:nch * FIN],
                        in_=vgd[:, pbase[b] * FIN:(pbase[b] + nch) * FIN])
                    oht = ohp.tile([P, (KT_M + 1) * P], fp8, tag="oht")
                    nc.gpsimd.dma_start(
                        out=oht[:, 0:nch * P],
                        in_=ohd[:, pbase[b] * P:(pbase[b] + nch) * P])
                    ps = pp.tile([P, FIN], f32, tag="agg")
                    for j in range(nch):
                        nc.tensor.matmul(
                            out=ps[:],
                            lhsT=oht[:, j * P:(j + 1) * P],
                            rhs=vg[:, j * FIN:(j + 1) * FIN],
                            start=(j == 0), stop=(j == nch - 1))
                    evict1(b, ps)

            nc.gpsimd.collective_compute(
                "AllGather", mybir.AluOpType.bypass, replica_groups=rg,
                ins=[u1b_b.opt()], outs=[u1f_b.opt()])

            # hT[0] = relu(w1_0^T x^T + b1_0) only needs x -- run it in the
            # shadow of the AllGather.
            with (
                tc.tile_pool(name="d0x", bufs=3) as xsp0,
                tc.tile_pool(name="d0p", bufs=3, space="PSUM") as hpp0,
            ):
                for f0 in range(0, NSH, 512):
                    w = min(512, NSH - f0)
                    xt = xsp0.tile([P, 512], bf16, tag="xs0")
                    nc.sync.dma_start(out=xt[:, 0:w], in_=xT[:, f0:f0 + w])
                    ph = hpp0.tile([P, 512], f32, tag="hps0")
                    nc.tensor.matmul(out=ph[:, 0:w], lhsT=w1_t[:, 0:H],
                                     rhs=xt[:, 0:w], start=True, stop=True)
                    nc.scalar.activation(
                        out=hT[0][:, f0:f0 + w], in_=ph[:, 0:w],
                        func=mybir.ActivationFunctionType.Relu,
                        bias=b1_t[:, 0:1], scale=1.0)

            # ================= P2: x2 = A x1 =================
            # hT1 dense + the layer-0/1 partial sums of [out1|z1] ride in
            # P2's shadow (tensor/ACT/DVE are far from saturated while the
            # Pool engine grinds through gather descriptor generation).
            with (
                tc.tile_pool(name="p2wa", bufs=A_LOOK + 2) as pwa,
                tc.tile_pool(name="p2wb", bufs=4) as pwb,
                tc.tile_pool(name="p2p", bufs=3, space="PSUM") as pp,
                tc.tile_pool(name="p2o", bufs=3) as ohp,
                tc.tile_pool(name="p2s", bufs=2) as sp,
                tc.tile_pool(name="p2e", bufs=3) as evp,
                tc.tile_pool(name="p2t", bufs=1, space="PSUM") as tpp,
                tc.tile_pool(name="p2h", bufs=2, space="PSUM") as hpp,
                tc.tile_pool(name="p2z", bufs=2, space="PSUM") as zop,
            ):
                for f0 in range(0, NSH, 512):
                    w = min(512, NSH - f0)
                    ph = hpp.tile([P, 512], f32, tag="hps")
                    nc.tensor.matmul(out=ph[:, 0:w], lhsT=w1_t[:, H:2 * H],
                                     rhs=x1T[:, f0:f0 + w],
                                     start=True, stop=True)
                    nc.scalar.activation(
                        out=hT[1][:, f0:f0 + w], in_=ph[:, 0:w],
                        func=mybir.ActivationFunctionType.Relu,
                        bias=b1_t[:, 1:2], scale=1.0)
                for b in range(BPC):
                    pz = zop.tile([P, 2 * CO], f32, tag="pz01")
                    for i in range(2):
                        nc.tensor.matmul(out=pz[:],
                                         lhsT=hT[i][:, b * P:(b + 1) * P],
                                         rhs=wc_t[i][:], start=(i == 0),
                                         stop=(i == 1))
                    nc.vector.tensor_copy(
                        out=work80[:, b * 2 * CO:(b + 1) * 2 * CO], in_=pz[:])

                def evict2(b, ps):
                    x2t = evp.tile([P, P], bf16, tag="x2t")
                    nc.scalar.mul(x2t[:], ps[:], dinvc_t[:, b:b + 1])
                    trp = tpp.tile([P, P], bf16, tag="trp2")
                    nc.tensor.transpose(out=trp[:], in_=x2t[:],
                                        identity=ident_t[:])
                    nc.vector.tensor_copy(out=x2T[:, b * P:(b + 1) * P],
                                          in_=trp[:])

                prop(u1f_a, u1f_b, u1b_a, u1b_b, FIN, evict2,
                     (pwa, pwb, pp, ohp, sp))

            # ========== mid: hT2 dense; finish [out1|z1]; chunked AG ==========
            with (
                tc.tile_pool(name="dps", bufs=3, space="PSUM") as hpp,
                tc.tile_pool(name="zps", bufs=3, space="PSUM") as zpp,
                tc.tile_pool(name="zev", bufs=3) as evp,
            ):
                for f0 in range(0, NSH, 512):
                    w = min(512, NSH - f0)
                    ph = hpp.tile([P, 512], f32, tag="hps")
                    nc.tensor.matmul(out=ph[:, 0:w], lhsT=w1_t[:, 2 * H:],
                                     rhs=x2T[:, f0:f0 + w],
                                     start=True, stop=True)
                    nc.scalar.activation(
                        out=hT[2][:, f0:f0 + w], in_=ph[:, 0:w],
                        func=mybir.ActivationFunctionType.Relu,
                        bias=b1_t[:, 2:3], scale=1.0)
                    for b in range(f0 // P, min((f0 + 512) // P, BPC)):
                        pz = zpp.tile([P, 2 * CO], f32, tag="pz2")
                        nc.tensor.matmul(out=pz[:],
                                         lhsT=hT[2][:, b * P:(b + 1) * P],
                                         rhs=wc_t[2][:], start=True,
                                         stop=True)
                        w80 = work80[:, b * 2 * CO:(b + 1) * 2 * CO]
                        nc.vector.tensor_tensor(out=w80, in0=w80, in1=pz[:],
                                                op=mybir.AluOpType.add)
                        uzt = evp.tile([P, P], bf16, tag="uzt")
                        nc.vector.memset(uzt[:, CO:P], 0)
                        nc.scalar.mul(uzt[:, 0:CO],
                                      work80[:, b * 2 * CO + CO:
                                             (b + 1) * 2 * CO],
                                      dinvc_t[:, b:b + 1])
                        nc.sync.dma_start(out=own_rows(uzb_a, uzb_b, b),
                                          in_=uzt[:])
                        if b == PA_BLK - 1:
                            nc.gpsimd.collective_compute(
                                "AllGather", mybir.AluOpType.bypass,
                                replica_groups=rg,
                                ins=[uzb_a.opt()], outs=[uzf_a.opt()])

                nc.gpsimd.collective_compute(
                    "AllGather", mybir.AluOpType.bypass, replica_groups=rg,
                    ins=[uzb_b.opt()], outs=[uzf_b.opt()])

            # ========== P3: out2 = dinv * A' z1, fused softmax head ==========
            mxn_all = res.tile([P, BPC], f32, tag="mxn_all")
            se_all = res.tile([P, BPC], f32, tag="se_all")
            lse = res.tile([P, BPC], f32, tag="lse")
            cb = res.tile([P, BPC], f32, tag="cb")
            LAG = 8
            BATCH = [(0, 24), (24, 48), (48, BPC)]  # lse/store batches
            with (
                tc.tile_pool(name="p3wa", bufs=A_LOOK + 2) as pwa,
                tc.tile_pool(name="p3wb", bufs=4) as pwb,
                tc.tile_pool(name="p3p", bufs=4, space="PSUM") as pp,
                tc.tile_pool(name="p3o", bufs=3) as ohp,
                tc.tile_pool(name="p3s", bufs=2) as sp,
                tc.tile_pool(name="p3f", bufs=4) as fp,
            ):
                def smax_exp(b):
                    ex = fp.tile([P, 2 * CO], f32, tag="ex")
                    nc.scalar.activation(
                        out=ex[:], in_=work80[:, b * 2 * CO:(b + 1) * 2 * CO],
                        func=mybir.ActivationFunctionType.Exp,
                        bias=mxn_all[:, b:b + 1], scale=-1.0,
                        accum_out=se_all[:, b:b + 1])

                def finish(lo, hi):
                    # r = t1 - mx - lse = -t1n + (mxn - lse)
                    nc.scalar.activation(
                        out=lse[:, lo:hi], in_=se_all[:, lo:hi],
                        func=mybir.ActivationFunctionType.Ln)
                    nc.vector.tensor_tensor(
                        out=cb[:, lo:hi], in0=mxn_all[:, lo:hi],
                        in1=lse[:, lo:hi], op=mybir.AluOpType.subtract)
                    for b in range(lo, hi):
                        r = fp.tile([P, 2 * CO], f32, tag="r")
                        nc.scalar.activation(
                            out=r[:],
                            in_=work80[:, b * 2 * CO:(b + 1) * 2 * CO],
                            func=mybir.ActivationFunctionType.Identity,
                            bias=cb[:, b:b + 1], scale=-1.0)
                        nc.sync.dma_start(out=out[b * P:(b + 1) * P, :],
                                          in_=r[:])

                def evict3(b, ps):
                    w80 = work80[:, b * 2 * CO:(b + 1) * 2 * CO]
                    nc.scalar.mul(
                        work80[:, b * 2 * CO + CO:(b + 1) * 2 * CO], ps[:],
                        dinvc_t[:, b:b + 1])
                    # t1n = -(logits + bias), in place over work80[b]
                    nc.vector.tensor_tensor(
                        out=w80, in0=brow_t[:], in1=w80,
                        op=mybir.AluOpType.subtract)
                    nc.vector.tensor_reduce(
                        out=mxn_all[:, b:b + 1], in_=w80,
                        axis=mybir.AxisListType.X, op=mybir.AluOpType.min)
                    if b >= LAG:
                        smax_exp(b - LAG)
                    for lo, hi in BATCH[:2]:
                        if b == hi + LAG - 1:
                            finish(lo, hi)

                prop(uzf_a, uzf_b, uzb_a, uzb_b, CO, evict3,
                     (pwa, pwb, pp, ohp, sp))
                for b in range(BPC - LAG, BPC):
                    smax_exp(b)
                finish(*BATCH[2])


def _get_compiled(K_a, K_b):
    key = (K_a, K_b)
    if key not in _COMPILED:
        nc = bacc.Bacc("TRN2", target_bir_lowering=False, debug=False,
                       num_devices=NCORES, num_swdge_queues=NQ)
        _build(nc, K_a, K_b)
        nc.compile()
        _COMPILED[key] = nc
    return _COMPILED[key]


def kernel(**inputs):
    global LAST_RESULT
    args = {k: np.asarray(v) for k, v in inputs.items()}
    in_maps, node2slot, K_a, K_b = _preprocess(
        args["x"].astype(np.float32), args["edge_index"],
        args["w1_0"].astype(np.float32), args["b1_0"].astype(np.float32),
        args["w1_1"].astype(np.float32), args["b1_1"].astype(np.float32),
        args["w1_2"].astype(np.float32), args["b1_2"].astype(np.float32),
        args["w2_0"].astype(np.float32), args["b2_0"].astype(np.float32),
        args["w2_1"].astype(np.float32), args["b2_1"].astype(np.float32),
    )
    nc = _get_compiled(K_a, K_b)
    res = run_bass_kernel_spmd(nc, in_maps, list(range(NCORES)))
    LAST_RESULT = res
    out_slot = np.concatenate([res.results[c]["out"] for c in range(NCORES)],
                              axis=0)
    return out_slot[node2slot].astype(np.float32)
